# revision 37
# baseline (speedup 1.0000x reference)
"""Trainium2 Bass kernel for a GPT-J-style (parallel-residual) decoder layer.

Problem: B=2, S=2048, D=1024, H=16 heads x 64, rotary_dim=16, FF=4096, causal.

Sharding (8 NeuronCores): data-parallel over batch (2) x tensor-parallel over
heads/FFN (4).  Core c handles batch c//4 and TP rank r=c%4: heads 4r..4r+3
(256 of the 1024 attention dims), FFN rows 1024r..1024r+1024.
LayerNorm affine params are folded into the weights on the host, so the device
computes a single normalized activation xhat shared by attention and FFN.
Each core returns partial^T = (attn_partial + ffn_partial)^T in [D, S] bf16;
the host sums the 4 TP partials per batch and adds x + b_o + b2.

Schedule (v1): fully pipelined.
  P1: per 4-tile group: DMA x -> LN stats -> batched sqrt -> xhat ->
      PE-transpose (identity matmul) -> QKV matmuls; rotary + PE-transpose of
      q,k to e-major per half.  No DRAM staging anywhere.
  P2/P3: attention is query-chunk-outer (sc = 512 cols); FFN1+GELU emitted
      after attention sc0 (single ACT-table swap Exp->Gelu->Exp);
      FFN2 and W_o accumulate into the SAME PSUM bank per (et, sc) right
      after each sc's heads finish, interleaved into the next sc's head loop;
      outputs stream to DRAM per 2-et chunk in bf16.
"""

import numpy as np
import ml_dtypes

import concourse.bass as bass
import concourse.mybir as mybir
import concourse.tile as tile
import concourse.bass_utils as bass_utils
from concourse import bacc
from concourse import masks
from concourse.bass import ds, ts

B, S, D = 2, 2048, 1024
H, HD = 16, 64
ROT, RH = 16, 8
FF = 4096
EPS = 1e-5
P = 128
NT = S // P            # 16 sequence tiles
DC = D // P            # 8 model-dim chunks
NH = 4                 # heads per core
DSH = NH * HD          # 256 attention dims per core
FSH = FF // 4          # 1024 FFN rows per core
NCORES = 8

F32 = mybir.dt.float32
BF16 = mybir.dt.bfloat16
F8 = mybir.dt.float8e4
DR = mybir.MatmulPerfMode.DoubleRow
AF = mybir.ActivationFunctionType
ALU = mybir.AluOpType
bf16 = ml_dtypes.bfloat16
f8e4 = ml_dtypes.float8_e4m3fn
WS = 64.0   # fp8 weight scale (power of 2; psum carries WS*value)


def _body(tc, aps, gelu_func):
    nc = tc.nc
    x_d = aps["x"].rearrange("(t p) d -> p t d", p=P)        # [128, 16, 1024]
    bqkv_d = aps["bqkv"]
    wo_d = aps["wo"]
    w1_d = aps["w1"]
    b1_d = aps["b1p"]
    w2_d = aps["w2"]
    cos_d = aps["cosr"]
    sin_d = aps["sinr"]
    mask_d = aps["maskd"]
    out_r = aps["outp"].rearrange("(c p) s -> p c s", p=P)   # [128, 8, 2048]

    with (
        tc.tile_pool(name="const", bufs=1) as const,
        tc.tile_pool(name="big", bufs=1) as big,
    ):
        # ---- persistent SBUF: weights + activations ----
        # x tiles stream first (alternating the two hwdge queues); weight
        # loads are emitted after so they don't delay the LN pipeline.
        # w1 is deferred until the FFN1 phase.
        xall = const.tile([P, NT, D], BF16)                  # 32KB/part
        wqkva_sb = const.tile([P, 4, 2, 512], F8)
        wqkvb_sb = const.tile([P, 4, 2, 256], F8)
        bqkv_sb = const.tile([P, 3 * DSH], F32)
        wo_sb = const.tile([P, DC, 2, P], F8)
        b1_sb = const.tile([P, DC], F32)
        cos_sb = const.tile([P, NT, RH], BF16)
        sin_sb = const.tile([P, NT, RH], BF16)
        mask_sb = const.tile([P, P], F8)
        w1_sb = const.tile([P, DC, DC, P], BF16)             # 16KB/part

        for t in range(NT):
            eng = nc.sync if t % 2 == 0 else nc.scalar
            eng.dma_start(xall[:, t, :], x_d[:, t, :])
            # weave the QKV-critical weights between the first x tiles on
            # the scalar hwdge queue so QKV/bias never wait on them
            if t == 1:
                nc.scalar.dma_start(wqkva_sb[:], aps["wqkva"])
            elif t == 3:
                nc.scalar.dma_start(wqkvb_sb[:], aps["wqkvb"])
            elif t == 5:
                nc.scalar.dma_start(bqkv_sb[:], bqkv_d)

        def load_weights():
            # weights not needed until FFN1 / rotary / stage C
            nc.scalar.dma_start(b1_sb[:], b1_d)
            for ft in range(DC):
                nc.scalar.dma_start(w1_sb[:, ft], w1_d[:, ft])
            nc.scalar.dma_start(wo_sb[:], wo_d)
            nc.scalar.dma_start(cos_sb[:], cos_d)
            nc.scalar.dma_start(sin_sb[:], sin_d)
            nc.scalar.dma_start(mask_sb[:], mask_d)
        eps_sb = const.tile([P, 1], F32)
        nc.vector.memset(eps_sb[:], EPS)
        ones_sb = const.tile([1, HD], BF16)
        nc.vector.memset(ones_sb[:], 1.0)
        ident = const.tile([P, P], BF16)
        masks.make_identity(nc, ident[:])
        mvall = const.tile([P, NT, 2], F32)                  # LN mean/var

        xhatT = big.tile([P, DC, S], BF16)          # xhat dim-major [d, s]
        xhatT8 = big.tile([P, 4, NT, 2, P], F8)     # fp8, kp-pair packed
        vp = big.tile([P, NT // 2, NH, 2, HD + 16], F8)  # v pair-packed+ones
        # inner width 80 = 16B-aligned even stride (dual-fp8 LDW restriction)
        qe = big.tile([P, 2, S], BF16)              # q e-major
        ke = big.tile([P, 2, S], BF16)              # k e-major
        ot = big.tile([P, 4, 2, 512], F8)           # attn out, sc-major
        hid = big.tile([P, DC, S], BF16)            # ffn hidden, f-major

        nc.vector.memset(vp[:, :, :, :, HD:HD + 1], 1.0)

        # ================= P1: LN + transpose + QKV + rotary =================
        with (
            tc.tile_pool(name="qkp", bufs=1) as qkp,
            tc.tile_pool(name="statp", bufs=8) as statp,
            tc.tile_pool(name="xhp", bufs=3) as xhp,
            tc.tile_pool(name="rotp", bufs=3) as rotp,
            tc.tile_pool(name="tpps", bufs=2, space="PSUM") as tpps,
            tc.tile_pool(name="qtps", bufs=1, space="PSUM") as qtps,
            tc.tile_pool(name="qaps", bufs=2, space="PSUM") as qaps,
            tc.tile_pool(name="qbps", bufs=1, space="PSUM") as qbps,
            tc.tile_pool(name="ff1ps", bufs=2, space="PSUM") as ff1ps,
        ):
            qk = qkp.tile([P, NT, 2 * DSH], BF16)   # q,k token-major (scoped)

            def emit_ffn1(sc):
                for ft in range(DC):
                    ps = ff1ps.tile([P, 512], F32, tag="f1")
                    for c in range(DC):
                        nc.tensor.matmul(ps[:], lhsT=w1_sb[:, ft, c, :],
                                         rhs=xhatT[:, c, ds(512 * sc, 512)],
                                         start=(c == 0), stop=(c == DC - 1))
                    nc.scalar.activation(hid[:, ft, ds(512 * sc, 512)],
                                         ps[:], gelu_func,
                                         bias=b1_sb[:, ft:ft + 1])

            def do_rotary(half):
                cosb = cos_sb[:, ds(8 * half, 8), :].unsqueeze(2) \
                    .to_broadcast([P, 8, NH, RH])
                sinb = sin_sb[:, ds(8 * half, 8), :].unsqueeze(2) \
                    .to_broadcast([P, 8, NH, RH])
                for part in range(2):   # 0: q, 1: k
                    sl = qk[:, ds(8 * half, 8), ds(DSH * part, DSH)].rearrange(
                        "p t (h e) -> p t h e", h=NH)
                    x1 = sl[:, :, :, 0:RH]
                    x2 = sl[:, :, :, RH:ROT]
                    t1 = rotp.tile([P, 8, NH, RH], BF16, tag="rt")
                    t2 = rotp.tile([P, 8, NH, RH], BF16, tag="rt")
                    t3 = rotp.tile([P, 8, NH, RH], BF16, tag="rt")
                    nc.vector.tensor_tensor(out=t1[:], in0=x1, in1=cosb,
                                            op=ALU.mult)
                    nc.vector.tensor_tensor(out=t2[:], in0=x2, in1=sinb,
                                            op=ALU.mult)
                    nc.vector.tensor_tensor(out=t1[:], in0=t1[:], in1=t2[:],
                                            op=ALU.subtract)
                    nc.vector.tensor_tensor(out=t2[:], in0=x1, in1=sinb,
                                            op=ALU.mult)
                    nc.vector.tensor_tensor(out=t3[:], in0=x2, in1=cosb,
                                            op=ALU.mult)
                    nc.vector.tensor_tensor(out=t2[:], in0=t2[:], in1=t3[:],
                                            op=ALU.add)
                    nc.vector.tensor_copy(out=x1, in_=t1[:])
                    nc.vector.tensor_copy(out=x2, in_=t2[:])

            def qk_transpose(t):
                # q,k of tile t -> e-major qe/ke columns [t*128, t*128+128)
                qt = qtps.tile([P, 4, P], BF16, tag="qt")
                for j in range(4):
                    nc.tensor.transpose(qt[:, j, :], qk[:, t, ds(P * j, P)],
                                        ident[:])
                nc.scalar.activation(qe[:, :, ts(t, P)], qt[:, 0:2, :],
                                     AF.Copy)
                nc.scalar.activation(ke[:, :, ts(t, P)], qt[:, 2:4, :],
                                     AF.Copy)

            for g in range(4):
                if g == 0:
                    # group 0 pipelines per-tile so tile 0's chain has the
                    # lowest scheduler priority (earliest execution)
                    rstds = []
                    def stats_tile(t):
                        st = statp.tile([P, 2, 6], F32, tag="st")
                        xr = xall[:, t, :].rearrange("p (a f) -> p a f",
                                                     f=512)
                        for sg in range(2):
                            nc.vector.bn_stats(st[:, sg, :], xr[:, sg, :])
                        nc.vector.bn_aggr(mvall[:, t, :], st[:])
                        sd = statp.tile([P, 1], F32, tag=f"s{t}")
                        nc.scalar.activation(sd[:], mvall[:, ds(t, 1), 1],
                                             AF.Sqrt, bias=eps_sb[:])
                        rs = statp.tile([P, 1], F32, tag=f"r{t}")
                        nc.vector.reciprocal(rs[:], sd[:])
                        rstds.append(rs)
                    rstd_of = lambda j: rstds[j][:, 0:1]
                else:
                    for j in range(4):
                        t = 4 * g + j
                        st = statp.tile([P, 2, 6], F32, tag="st")
                        xr = xall[:, t, :].rearrange("p (a f) -> p a f",
                                                     f=512)
                        for sg in range(2):
                            nc.vector.bn_stats(st[:, sg, :], xr[:, sg, :])
                        nc.vector.bn_aggr(mvall[:, t, :], st[:])
                    stdg = statp.tile([P, 4], F32, tag="sd", name=f"sd{g}")
                    nc.scalar.activation(stdg[:], mvall[:, ds(4 * g, 4), 1],
                                         AF.Sqrt, bias=eps_sb[:])
                    rstdg = statp.tile([P, 4], F32, tag="rs", name=f"rs{g}")
                    nc.vector.reciprocal(rstdg[:], stdg[:])
                    rstd_of = lambda j, r=rstdg: r[:, ds(j, 1)]
                for j in range(4):
                    t = 4 * g + j
                    if g == 0:
                        stats_tile(t)
                        if j == 3:
                            load_weights()
                    xh = xhp.tile([P, D], BF16, tag="xh")
                    nc.vector.tensor_scalar(out=xh[:], in0=xall[:, t, :],
                                            scalar1=mvall[:, t, 0:1],
                                            scalar2=rstd_of(j),
                                            op0=ALU.subtract, op1=ALU.mult)
                    tp = tpps.tile([P, DC, P], BF16, tag="tp")
                    for c in range(DC):
                        nc.tensor.transpose(tp[:, c, :], xh[:, ds(P * c, P)],
                                            ident[:])
                    nc.scalar.activation(
                        xhatT8[:, :, t, :, :],
                        tp[:].rearrange("p (a b) c -> p a b c", b=2),
                        AF.Copy)
                    nc.vector.tensor_copy(out=xhatT[:, :, ts(t, P)], in_=tp[:])
                    # QKV for tile t (fp8 DoubleRow: K=256 per pass)
                    psa = qaps.tile([P, 512], F32, tag="qa")
                    psb = qbps.tile([P, 256], F32, tag="qb")
                    for kp in range(4):
                        l = xhatT8[:, kp, t, :, :]
                        nc.tensor.matmul(psa[:], lhsT=l,
                                         rhs=wqkva_sb[:, kp],
                                         start=(kp == 0), stop=(kp == 3),
                                         perf_mode=DR)
                        nc.tensor.matmul(psb[:], lhsT=l,
                                         rhs=wqkvb_sb[:, kp],
                                         start=(kp == 0), stop=(kp == 3),
                                         perf_mode=DR)
                    nc.vector.scalar_tensor_tensor(
                        out=qk[:, t, :], in0=psa[:], scalar=1.0 / WS,
                        in1=bqkv_sb[:, 0:512], op0=ALU.mult, op1=ALU.add)
                    nc.vector.scalar_tensor_tensor(
                        out=vp[:, t // 2, :, t % 2, 0:HD],
                        in0=psb[:].rearrange("p (h e) -> p h e", h=NH),
                        scalar=1.0 / WS,
                        in1=bqkv_sb[:, 512:768].rearrange(
                            "p (h e) -> p h e", h=NH),
                        op0=ALU.mult, op1=ALU.add)
                if g % 2 == 1:
                    half = g // 2
                    do_rotary(half)
                    for t in range(8 * half, 8 * half + 8):
                        qk_transpose(t)
                emit_ffn1(g)

        # ============ P2/P3: attention (sc-outer) + FFN1 + fused FFN2/WO ====
        with (
            tc.tile_pool(name="w2p", bufs=1) as w2p,
            tc.tile_pool(name="ptp", bufs=3) as ptp,
            tc.tile_pool(name="sump", bufs=2) as sump,
            tc.tile_pool(name="obp", bufs=4) as obp,
            tc.tile_pool(name="scps", bufs=2, space="PSUM") as scps,
            tc.tile_pool(name="ovps", bufs=2, space="PSUM") as ovps,
            tc.tile_pool(name="fwps", bufs=2, space="PSUM") as fwps,
        ):
            w2_sb = w2p.tile([P, DC, DC, P], BF16)           # 16KB/part
            for et in range(DC):
                nc.sync.dma_start(w2_sb[:, et], w2_d[:, et])

            def emit_head(sc, h):
                """Attention for query cols [512sc, 512sc+512), head h.

                Key tiles are processed in pairs (i0, i1 = 2k, 2k+1): exp
                probabilities go to the two fp8 planes of pt2 and one
                DoubleRow matmul contracts both tiles (256 keys) at once.
                """
                base = HD * (h % 2)
                cix = h // 2
                ov = ovps.tile([HD + 1, 512], F32, tag="ov",
                               name=f"ov_{sc}_{h}")
                klast = 2 * sc + 1
                for k in range(klast + 1):
                    i0, i1 = 2 * k, 2 * k + 1
                    lo0 = max(512 * sc, P * i0)
                    lo1 = max(512 * sc, P * i1)
                    wid0 = 512 * (sc + 1) - lo0      # 512 or 256
                    wid1 = 512 * (sc + 1) - lo1
                    o0 = lo0 - 512 * sc              # ov-relative offset
                    rel1 = lo1 - lo0                 # plane-1 start in pt2
                    pt2 = ptp.tile([P, 2, wid0], F8, tag=f"pt{wid0}")
                    for j, (i, lo, wid, rel) in enumerate(
                            ((i0, lo0, wid0, 0), (i1, lo1, wid1, rel1))):
                        ps = scps.tile([P, 512], F32, tag="sc")
                        nc.tensor.matmul(
                            ps[:, :wid],
                            lhsT=ke[base:base + HD, cix, ts(i, P)],
                            rhs=qe[base:base + HD, cix, ds(lo, wid)],
                            start=True, stop=True)
                        nc.scalar.activation(pt2[:, j, ds(rel, wid)],
                                             ps[:, :wid], AF.Exp, scale=0.125)
                        if lo == P * i:   # diagonal block: causal mask
                            nc.vector.tensor_tensor(
                                out=pt2[:, j, ds(rel, P)],
                                in0=pt2[:, j, ds(rel, P)],
                                in1=mask_sb[:], op=ALU.mult)
                    if rel1 > 0:   # plane 1 has no keys for cols [lo0, lo1)
                        nc.vector.memset(pt2[:, 1, ds(0, rel1)], 0.0)
                    nc.tensor.matmul(
                        ov[:, ds(o0, wid0)],
                        lhsT=vp[:, k, h, :, 0:HD + 1],
                        rhs=pt2[:],
                        start=(k == 0), stop=(k == klast),
                        perf_mode=DR)
                # renormalize: ot = ov[:HD] * (1/rowsum) broadcast via PE
                dst = ot[base:base + HD, sc, cix, :]
                otmp = sump.tile([P, 512], BF16, tag="ot", name=f"ot_{sc}_{h}")
                nc.vector.tensor_copy(out=otmp[base:base + HD, :],
                                      in_=ov[0:HD, :])
                sume = sump.tile([1, 512], F32, tag="se", name=f"se_{sc}_{h}")
                nc.vector.tensor_copy(out=sume[:], in_=ov[HD:HD + 1, :])
                rinv = sump.tile([1, 512], F32, tag="ri", name=f"ri_{sc}_{h}")
                nc.vector.reciprocal_approx_fast(out=rinv[:], in_=sume[:])
                rinv_bf = sump.tile([1, 512], BF16, tag="rb",
                                    name=f"rb_{sc}_{h}")
                nc.vector.tensor_copy(out=rinv_bf[:], in_=rinv[:])
                rbp = ovps.tile([P, 512], F32, tag="ov", name=f"rp_{sc}_{h}")
                nc.tensor.matmul(rbp[base:base + HD, :], lhsT=ones_sb[:],
                                 rhs=rinv_bf[:], start=True, stop=True)
                nc.vector.tensor_tensor(out=dst, in0=otmp[base:base + HD, :],
                                        in1=rbp[base:base + HD, :],
                                        op=ALU.mult)

            def emit_fw(sc, ets):
                """Fused FFN2 + W_o for out chunks ets at query cols sc.

                W2 is host-scaled by WS (bf16, exact) and W_o is fp8 scaled
                by WS, so both accumulate WS*value into the same PSUM bank;
                the final copy multiplies by 1/WS.
                """
                ob = obp.tile([P, 2, 512], BF16, tag="ob")
                for k, et in enumerate(ets):
                    po = fwps.tile([P, 512], F32, tag="fw")
                    for c in range(DC):
                        nc.tensor.matmul(po[:], lhsT=w2_sb[:, et, c, :],
                                         rhs=hid[:, c, ds(512 * sc, 512)],
                                         start=(c == 0), stop=False)
                    nc.tensor.matmul(po[:], lhsT=wo_sb[:, et],
                                     rhs=ot[:, sc],
                                     start=False, stop=True, perf_mode=DR)
                    nc.vector.tensor_scalar_mul(ob[:, k, :], po[:],
                                                 1.0 / WS)
                nc.sync.dma_start(
                    out_r[:, ds(ets[0], 2), ds(512 * sc, 512)], ob[:])

            # all of FFN1 first: keeps every Gelu ACTIVATE ahead (in scheduler
            # priority) of every attention Exp, so the ACT table swaps once.

            # attention; FW(sc-1) interleaved into sc's head loop
            for sc in range(4):
                for h in range(NH):
                    emit_head(sc, h)
                    if sc > 0:
                        emit_fw(sc - 1, [2 * h, 2 * h + 1])
            for j in range(4):
                emit_fw(3, [2 * j, 2 * j + 1])


def build(gelu_func=None):
    if gelu_func is None:
        gelu_func = AF.Gelu
    nc = bacc.Bacc("TRN2", target_bir_lowering=False, debug=False,
                   enable_asserts=True, num_devices=NCORES)
    aps = {}

    def din(name, shape, dtype):
        aps[name] = nc.dram_tensor(name, list(shape), dtype,
                                   kind="ExternalInput").ap()

    din("x", (S, D), BF16)
    din("wqkva", (P, 4, 2, 512), F8)
    din("wqkvb", (P, 4, 2, 256), F8)
    din("bqkv", (P, 3 * DSH), F32)
    din("wo", (P, DC, 2, P), F8)
    din("w1", (P, DC, DC, P), BF16)
    din("b1p", (P, DC), F32)
    din("w2", (P, DC, DC, P), BF16)
    din("cosr", (P, NT, RH), BF16)
    din("sinr", (P, NT, RH), BF16)
    din("maskd", (P, P), F8)
    aps["outp"] = nc.dram_tensor("outp", [D, S], BF16,
                                 kind="ExternalOutput").ap()

    with tile.TileContext(nc) as tc:
        _body(tc, aps, gelu_func)
    nc.compile()
    return nc


def make_in_maps(inputs):
    x = np.asarray(inputs["x"], np.float32)
    Wqkv = np.asarray(inputs["W_qkv"], np.float32)
    b_qkv = np.asarray(inputs["b_qkv"], np.float32)
    Wo = np.asarray(inputs["W_o"], np.float32)
    ln1w = np.asarray(inputs["ln1_w"], np.float32)
    ln1b = np.asarray(inputs["ln1_b"], np.float32)
    ln2w = np.asarray(inputs["ln2_w"], np.float32)
    ln2b = np.asarray(inputs["ln2_b"], np.float32)
    W1 = np.asarray(inputs["W1"], np.float32)
    b1 = np.asarray(inputs["b1"], np.float32)
    W2 = np.asarray(inputs["W2"], np.float32)
    freqs = np.asarray(inputs["freqs_cis"], np.float32)

    cos = freqs[0, 0, :, :, 0]
    sin = freqs[0, 0, :, :, 1]
    cosr = np.ascontiguousarray(
        cos.reshape(NT, P, RH).transpose(1, 0, 2)).astype(bf16)
    sinr = np.ascontiguousarray(
        sin.reshape(NT, P, RH).transpose(1, 0, 2)).astype(bf16)
    kq = np.arange(P)
    maskd = (kq[:, None] <= kq[None, :]).astype(f8e4)

    in_maps = []
    for core in range(NCORES):
        b = core // 4
        r = core % 4
        sl = slice(256 * r, 256 * r + 256)
        Ws = np.concatenate([Wqkv[0:D][sl], Wqkv[D:2 * D][sl],
                             Wqkv[2 * D:3 * D][sl]], 0)          # [768, 1024]
        bq = np.concatenate([b_qkv[0:D][sl], b_qkv[D:2 * D][sl],
                             b_qkv[2 * D:3 * D][sl]], 0)
        Wsp = Ws * ln1w[None, :]
        bqp = (bq + Ws @ ln1b).astype(np.float32)
        # fp8 DoubleRow layout [p, kp, j, out]: contraction d = (2kp+j)*128+p
        wq8 = (Wsp.T * 64.0).reshape(4, 2, P, 3 * DSH).transpose(2, 0, 1, 3)
        wqkva_l = np.ascontiguousarray(wq8[:, :, :, 0:512]).astype(f8e4)
        wqkvb_l = np.ascontiguousarray(wq8[:, :, :, 512:768]).astype(f8e4)
        bqkv_l = np.ascontiguousarray(
            np.broadcast_to(bqp[None, :], (P, 3 * DSH))).astype(np.float32)
        Wos = Wo[:, sl]                                           # [1024, 256]
        wo_l = np.ascontiguousarray(
            (Wos.T * 64.0).reshape(2, P, DC, P)
            .transpose(1, 2, 0, 3)).astype(f8e4)
        W1s = W1[FSH * r: FSH * (r + 1)]                          # [1024, 1024]
        W1p = W1s * ln2w[None, :]
        b1p = (b1[FSH * r: FSH * (r + 1)] + W1s @ ln2b).astype(np.float32)
        w1_l = np.ascontiguousarray(
            W1p.reshape(DC, P, DC, P).transpose(3, 0, 2, 1)).astype(bf16)
        b1_l = np.ascontiguousarray(b1p.reshape(DC, P).T).astype(np.float32)
        W2s = W2[:, FSH * r: FSH * (r + 1)] * 64.0                # [1024, 1024]
        w2_l = np.ascontiguousarray(
            W2s.reshape(DC, P, DC, P).transpose(3, 0, 2, 1)).astype(bf16)
        in_maps.append(dict(
            x=np.ascontiguousarray(x[b]).astype(bf16),
            wqkva=wqkva_l, wqkvb=wqkvb_l,
            bqkv=bqkv_l, wo=wo_l,
            w1=w1_l, b1p=b1_l, w2=w2_l, cosr=cosr, sinr=sinr, maskd=maskd))
    return in_maps


def gather(inputs, results):
    x = np.asarray(inputs["x"], np.float32)
    bias = (np.asarray(inputs["b_o"], np.float32)
            + np.asarray(inputs["b2"], np.float32))
    outs = [np.asarray(res["outp"], np.float32) for res in results]
    out = np.empty((B, S, D), np.float32)
    for b in range(B):
        acc = outs[4 * b] + outs[4 * b + 1] + outs[4 * b + 2] + outs[4 * b + 3]
        out[b] = x[b] + acc.T + bias[None, :]
    return out


_CACHE = {}


def kernel(**inputs):
    if "nc" not in _CACHE:
        _CACHE["nc"] = build()
    nc = _CACHE["nc"]
    in_maps = make_in_maps(inputs)
    res = bass_utils.run_bass_kernel_spmd(nc, in_maps,
                                          core_ids=list(range(NCORES)))
    return gather(inputs, res.results)


# revision 38
# speedup vs baseline: 1.0392x; 1.0392x over previous
"""Trainium2 Bass kernel for a GPT-J-style (parallel-residual) decoder layer.

Problem: B=2, S=2048, D=1024, H=16 heads x 64, rotary_dim=16, FF=4096, causal.

Sharding (8 NeuronCores): data-parallel over batch (2) x tensor-parallel over
heads/FFN (4).  Core c handles batch c//4 and TP rank r=c%4: heads 4r..4r+3
(256 of the 1024 attention dims), FFN rows 1024r..1024r+1024.
LayerNorm affine params are folded into the weights on the host, so the device
computes a single normalized activation xhat shared by attention and FFN.
Each core returns partial^T = (attn_partial + ffn_partial)^T in [D, S] bf16;
the host sums the 4 TP partials per batch and adds x + b_o + b2.

Schedule (v1): fully pipelined.
  P1: per 4-tile group: DMA x -> LN stats -> batched sqrt -> xhat ->
      PE-transpose (identity matmul) -> QKV matmuls; rotary + PE-transpose of
      q,k to e-major per half.  No DRAM staging anywhere.
  P2/P3: attention is query-chunk-outer (sc = 512 cols); FFN1+GELU emitted
      after attention sc0 (single ACT-table swap Exp->Gelu->Exp);
      FFN2 and W_o accumulate into the SAME PSUM bank per (et, sc) right
      after each sc's heads finish, interleaved into the next sc's head loop;
      outputs stream to DRAM per 2-et chunk in bf16.
"""

import numpy as np
import ml_dtypes

import concourse.bass as bass
import concourse.mybir as mybir
import concourse.tile as tile
import concourse.bass_utils as bass_utils
from concourse import bacc
from concourse import masks
from concourse.bass import ds, ts

B, S, D = 2, 2048, 1024
H, HD = 16, 64
ROT, RH = 16, 8
FF = 4096
EPS = 1e-5
P = 128
NT = S // P            # 16 sequence tiles
DC = D // P            # 8 model-dim chunks
NH = 4                 # heads per core
DSH = NH * HD          # 256 attention dims per core
FSH = FF // 4          # 1024 FFN rows per core
NCORES = 8

F32 = mybir.dt.float32
BF16 = mybir.dt.bfloat16
F8 = mybir.dt.float8e4
DR = mybir.MatmulPerfMode.DoubleRow
AF = mybir.ActivationFunctionType
ALU = mybir.AluOpType
bf16 = ml_dtypes.bfloat16
f8e4 = ml_dtypes.float8_e4m3fn
WS = 64.0   # fp8 weight scale (power of 2; psum carries WS*value)


def _body(tc, aps, gelu_func):
    nc = tc.nc
    x_d = aps["x"].rearrange("(t p) d -> p t d", p=P)        # [128, 16, 1024]
    bqkv_d = aps["bqkv"]
    wo_d = aps["wo"]
    w1_d = aps["w1"]
    b1_d = aps["b1p"]
    w2_d = aps["w2"]
    cos_d = aps["cosr"]
    sin_d = aps["sinr"]
    mask_d = aps["maskd"]
    out_r = aps["outp"].rearrange("(c p) s -> p c s", p=P)   # [128, 8, 2048]

    with (
        tc.tile_pool(name="const", bufs=1) as const,
        tc.tile_pool(name="big", bufs=1) as big,
    ):
        # ---- persistent SBUF: weights + activations ----
        # x tiles stream first (alternating the two hwdge queues); weight
        # loads are emitted after so they don't delay the LN pipeline.
        # w1 is deferred until the FFN1 phase.
        xall = const.tile([P, NT, D], BF16)                  # 32KB/part
        wqkva_sb = const.tile([P, 4, 2, 512], F8)
        wqkvb_sb = const.tile([P, 4, 2, 256], F8)
        bqkv_sb = const.tile([P, 3 * DSH], F32)
        wo_sb = const.tile([P, DC, 2, P], F8)
        b1_sb = const.tile([P, DC], F32)
        cos_sb = const.tile([P, NT, RH], BF16)
        sin_sb = const.tile([P, NT, RH], BF16)
        mask_sb = const.tile([P, P], F8)
        w1_sb = const.tile([P, DC, DC, P], BF16)             # 16KB/part

        for t in range(NT):
            eng = nc.sync if t % 2 == 0 else nc.scalar
            eng.dma_start(xall[:, t, :], x_d[:, t, :])
            # weave the QKV-critical weights between the first x tiles on
            # the scalar hwdge queue so QKV/bias never wait on them
            if t == 1:
                nc.scalar.dma_start(wqkva_sb[:], aps["wqkva"])
            elif t == 3:
                nc.scalar.dma_start(wqkvb_sb[:], aps["wqkvb"])
            elif t == 5:
                nc.scalar.dma_start(bqkv_sb[:], bqkv_d)

        def load_weights():
            # weights not needed until FFN1 / rotary / stage C
            nc.scalar.dma_start(b1_sb[:], b1_d)
            for ft in range(DC):
                nc.scalar.dma_start(w1_sb[:, ft], w1_d[:, ft])
            nc.scalar.dma_start(wo_sb[:], wo_d)
            nc.scalar.dma_start(cos_sb[:], cos_d)
            nc.scalar.dma_start(sin_sb[:], sin_d)
            nc.scalar.dma_start(mask_sb[:], mask_d)
        eps_sb = const.tile([P, 1], F32)
        nc.vector.memset(eps_sb[:], EPS)
        ones_sb = const.tile([1, HD], BF16)
        nc.vector.memset(ones_sb[:], 1.0)
        ident = const.tile([P, P], BF16)
        masks.make_identity(nc, ident[:])
        mvall = const.tile([P, NT, 2], F32)                  # LN mean/var

        xhatT = big.tile([P, DC, S], BF16)          # xhat dim-major [d, s]
        xhatT8 = big.tile([P, 4, NT, 2, P], F8)     # fp8, kp-pair packed
        vp = big.tile([P, NT // 2, NH, 2, HD + 16], F8)  # v pair-packed+ones
        # inner width 80 = 16B-aligned even stride (dual-fp8 LDW restriction)
        qe = big.tile([P, 2, S], BF16)              # q e-major
        ke = big.tile([P, 2, S], BF16)              # k e-major
        ot = big.tile([P, 4, 2, 512], F8)           # attn out, sc-major
        hid = big.tile([P, DC, S], BF16)            # ffn hidden, f-major

        nc.vector.memset(vp[:, :, :, :, HD:HD + 1], 1.0)

        # ================= P1: LN + transpose + QKV + rotary =================
        with (
            tc.tile_pool(name="qkp", bufs=1) as qkp,
            tc.tile_pool(name="statp", bufs=8) as statp,
            tc.tile_pool(name="xhp", bufs=3) as xhp,
            tc.tile_pool(name="rotp", bufs=3) as rotp,
            tc.tile_pool(name="tpps", bufs=2, space="PSUM") as tpps,
            tc.tile_pool(name="qtps", bufs=1, space="PSUM") as qtps,
            tc.tile_pool(name="qaps", bufs=2, space="PSUM") as qaps,
            tc.tile_pool(name="qbps", bufs=1, space="PSUM") as qbps,
            tc.tile_pool(name="ff1ps", bufs=2, space="PSUM") as ff1ps,
        ):
            qk = qkp.tile([P, NT, 2 * DSH], BF16)   # q,k token-major (scoped)

            def emit_ffn1(sc):
                for ft in range(DC):
                    ps = ff1ps.tile([P, 512], F32, tag="f1")
                    for c in range(DC):
                        nc.tensor.matmul(ps[:], lhsT=w1_sb[:, ft, c, :],
                                         rhs=xhatT[:, c, ds(512 * sc, 512)],
                                         start=(c == 0), stop=(c == DC - 1))
                    nc.scalar.activation(hid[:, ft, ds(512 * sc, 512)],
                                         ps[:], gelu_func,
                                         bias=b1_sb[:, ft:ft + 1])

            def do_rotary(half):
                cosb = cos_sb[:, ds(8 * half, 8), :].unsqueeze(2) \
                    .to_broadcast([P, 8, NH, RH])
                sinb = sin_sb[:, ds(8 * half, 8), :].unsqueeze(2) \
                    .to_broadcast([P, 8, NH, RH])
                for part in range(2):   # 0: q, 1: k
                    sl = qk[:, ds(8 * half, 8), ds(DSH * part, DSH)].rearrange(
                        "p t (h e) -> p t h e", h=NH)
                    x1 = sl[:, :, :, 0:RH]
                    x2 = sl[:, :, :, RH:ROT]
                    t1 = rotp.tile([P, 8, NH, RH], BF16, tag="rt")
                    t2 = rotp.tile([P, 8, NH, RH], BF16, tag="rt")
                    t3 = rotp.tile([P, 8, NH, RH], BF16, tag="rt")
                    nc.vector.tensor_tensor(out=t1[:], in0=x1, in1=cosb,
                                            op=ALU.mult)
                    nc.vector.tensor_tensor(out=t2[:], in0=x2, in1=sinb,
                                            op=ALU.mult)
                    nc.vector.tensor_tensor(out=t1[:], in0=t1[:], in1=t2[:],
                                            op=ALU.subtract)
                    nc.vector.tensor_tensor(out=t2[:], in0=x1, in1=sinb,
                                            op=ALU.mult)
                    nc.vector.tensor_tensor(out=t3[:], in0=x2, in1=cosb,
                                            op=ALU.mult)
                    nc.vector.tensor_tensor(out=t2[:], in0=t2[:], in1=t3[:],
                                            op=ALU.add)
                    nc.vector.tensor_copy(out=x1, in_=t1[:])
                    nc.vector.tensor_copy(out=x2, in_=t2[:])

            def qk_transpose(t):
                # q,k of tile t -> e-major qe/ke columns [t*128, t*128+128)
                qt = qtps.tile([P, 4, P], BF16, tag="qt")
                for j in range(4):
                    nc.tensor.transpose(qt[:, j, :], qk[:, t, ds(P * j, P)],
                                        ident[:])
                nc.scalar.activation(qe[:, :, ts(t, P)], qt[:, 0:2, :],
                                     AF.Copy)
                nc.scalar.activation(ke[:, :, ts(t, P)], qt[:, 2:4, :],
                                     AF.Copy)

            for g in range(4):
                if g == 0:
                    # group 0 pipelines per-tile so tile 0's chain has the
                    # lowest scheduler priority (earliest execution)
                    rstds = []
                    def stats_tile(t):
                        st = statp.tile([P, 2, 6], F32, tag="st")
                        xr = xall[:, t, :].rearrange("p (a f) -> p a f",
                                                     f=512)
                        for sg in range(2):
                            nc.vector.bn_stats(st[:, sg, :], xr[:, sg, :])
                        nc.vector.bn_aggr(mvall[:, t, :], st[:])
                        sd = statp.tile([P, 1], F32, tag=f"s{t}")
                        nc.scalar.activation(sd[:], mvall[:, ds(t, 1), 1],
                                             AF.Sqrt, bias=eps_sb[:])
                        rs = statp.tile([P, 1], F32, tag=f"r{t}")
                        nc.vector.reciprocal(rs[:], sd[:])
                        rstds.append(rs)
                    rstd_of = lambda j: rstds[j][:, 0:1]
                else:
                    for j in range(4):
                        t = 4 * g + j
                        st = statp.tile([P, 2, 6], F32, tag="st")
                        xr = xall[:, t, :].rearrange("p (a f) -> p a f",
                                                     f=512)
                        for sg in range(2):
                            nc.vector.bn_stats(st[:, sg, :], xr[:, sg, :])
                        nc.vector.bn_aggr(mvall[:, t, :], st[:])
                    stdg = statp.tile([P, 4], F32, tag="sd", name=f"sd{g}")
                    nc.scalar.activation(stdg[:], mvall[:, ds(4 * g, 4), 1],
                                         AF.Sqrt, bias=eps_sb[:])
                    rstdg = statp.tile([P, 4], F32, tag="rs", name=f"rs{g}")
                    nc.vector.reciprocal(rstdg[:], stdg[:])
                    rstd_of = lambda j, r=rstdg: r[:, ds(j, 1)]
                for j in range(4):
                    t = 4 * g + j
                    if g == 0:
                        stats_tile(t)
                        if j == 3:
                            load_weights()
                    xh = xhp.tile([P, D], BF16, tag="xh")
                    nc.vector.tensor_scalar(out=xh[:], in0=xall[:, t, :],
                                            scalar1=mvall[:, t, 0:1],
                                            scalar2=rstd_of(j),
                                            op0=ALU.subtract, op1=ALU.mult)
                    tp = tpps.tile([P, DC, P], BF16, tag="tp")
                    for c in range(DC):
                        nc.tensor.transpose(tp[:, c, :], xh[:, ds(P * c, P)],
                                            ident[:])
                    nc.scalar.activation(
                        xhatT8[:, :, t, :, :],
                        tp[:].rearrange("p (a b) c -> p a b c", b=2),
                        AF.Copy)
                    nc.vector.tensor_copy(out=xhatT[:, :, ts(t, P)], in_=tp[:])
                    # QKV for tile t (fp8 DoubleRow: K=256 per pass)
                    psa = qaps.tile([P, 512], F32, tag="qa")
                    psb = qbps.tile([P, 256], F32, tag="qb")
                    for kp in range(4):
                        l = xhatT8[:, kp, t, :, :]
                        nc.tensor.matmul(psa[:], lhsT=l,
                                         rhs=wqkva_sb[:, kp],
                                         start=(kp == 0), stop=(kp == 3),
                                         perf_mode=DR)
                        nc.tensor.matmul(psb[:], lhsT=l,
                                         rhs=wqkvb_sb[:, kp],
                                         start=(kp == 0), stop=(kp == 3),
                                         perf_mode=DR)
                    nc.vector.scalar_tensor_tensor(
                        out=qk[:, t, :], in0=psa[:], scalar=1.0 / WS,
                        in1=bqkv_sb[:, 0:512], op0=ALU.mult, op1=ALU.add)
                    nc.vector.scalar_tensor_tensor(
                        out=vp[:, t // 2, :, t % 2, 0:HD],
                        in0=psb[:].rearrange("p (h e) -> p h e", h=NH),
                        scalar=1.0 / WS,
                        in1=bqkv_sb[:, 512:768].rearrange(
                            "p (h e) -> p h e", h=NH),
                        op0=ALU.mult, op1=ALU.add)
                if g % 2 == 1:
                    half = g // 2
                    do_rotary(half)
                    for t in range(8 * half, 8 * half + 8):
                        qk_transpose(t)
                emit_ffn1(g)

        # ============ P2/P3: attention (sc-outer) + FFN1 + fused FFN2/WO ====
        with (
            tc.tile_pool(name="w2p", bufs=1) as w2p,
            tc.tile_pool(name="ptp", bufs=4) as ptp,
            tc.tile_pool(name="sump", bufs=2) as sump,
            tc.tile_pool(name="obp", bufs=4) as obp,
            tc.tile_pool(name="scps", bufs=3, space="PSUM") as scps,
            tc.tile_pool(name="ovps", bufs=2, space="PSUM") as ovps,
            tc.tile_pool(name="fwps", bufs=3, space="PSUM") as fwps,
        ):
            w2_sb = w2p.tile([P, DC, DC, P], BF16)           # 16KB/part
            for et in range(DC):
                nc.sync.dma_start(w2_sb[:, et], w2_d[:, et])

            def emit_head(sc, h):
                """Attention for query cols [512sc, 512sc+512), head h.

                Key tiles are processed in pairs (i0, i1 = 2k, 2k+1): exp
                probabilities go to the two fp8 planes of pt2 and one
                DoubleRow matmul contracts both tiles (256 keys) at once.
                """
                base = HD * (h % 2)
                cix = h // 2
                ov = ovps.tile([HD + 1, 512], F32, tag="ov",
                               name=f"ov_{sc}_{h}")
                klast = 2 * sc + 1
                for k in range(klast + 1):
                    i0, i1 = 2 * k, 2 * k + 1
                    lo0 = max(512 * sc, P * i0)
                    lo1 = max(512 * sc, P * i1)
                    wid0 = 512 * (sc + 1) - lo0      # 512 or 256
                    wid1 = 512 * (sc + 1) - lo1
                    o0 = lo0 - 512 * sc              # ov-relative offset
                    rel1 = lo1 - lo0                 # plane-1 start in pt2
                    pt2 = ptp.tile([P, 2, wid0], F8, tag=f"pt{wid0}")
                    for j, (i, lo, wid, rel) in enumerate(
                            ((i0, lo0, wid0, 0), (i1, lo1, wid1, rel1))):
                        ps = scps.tile([P, 512], F32, tag="sc")
                        nc.tensor.matmul(
                            ps[:, :wid],
                            lhsT=ke[base:base + HD, cix, ts(i, P)],
                            rhs=qe[base:base + HD, cix, ds(lo, wid)],
                            start=True, stop=True)
                        nc.scalar.activation(pt2[:, j, ds(rel, wid)],
                                             ps[:, :wid], AF.Exp, scale=0.125)
                        if lo == P * i:   # diagonal block: causal mask
                            nc.vector.tensor_tensor(
                                out=pt2[:, j, ds(rel, P)],
                                in0=pt2[:, j, ds(rel, P)],
                                in1=mask_sb[:], op=ALU.mult)
                    if rel1 > 0:   # plane 1 has no keys for cols [lo0, lo1)
                        nc.vector.memset(pt2[:, 1, ds(0, rel1)], 0.0)
                    nc.tensor.matmul(
                        ov[:, ds(o0, wid0)],
                        lhsT=vp[:, k, h, :, 0:HD + 1],
                        rhs=pt2[:],
                        start=(k == 0), stop=(k == klast),
                        perf_mode=DR)
                # renormalize: ot = ov[:HD] * (1/rowsum) broadcast via PE
                dst = ot[base:base + HD, sc, cix, :]
                otmp = sump.tile([P, 512], BF16, tag="ot", name=f"ot_{sc}_{h}")
                nc.vector.tensor_copy(out=otmp[base:base + HD, :],
                                      in_=ov[0:HD, :])
                sume = sump.tile([1, 512], F32, tag="se", name=f"se_{sc}_{h}")
                nc.vector.tensor_copy(out=sume[:], in_=ov[HD:HD + 1, :])
                rinv = sump.tile([1, 512], F32, tag="ri", name=f"ri_{sc}_{h}")
                nc.vector.reciprocal_approx_fast(out=rinv[:], in_=sume[:])
                rinv_bf = sump.tile([1, 512], BF16, tag="rb",
                                    name=f"rb_{sc}_{h}")
                nc.vector.tensor_copy(out=rinv_bf[:], in_=rinv[:])
                rbp = ovps.tile([P, 512], F32, tag="ov", name=f"rp_{sc}_{h}")
                nc.tensor.matmul(rbp[base:base + HD, :], lhsT=ones_sb[:],
                                 rhs=rinv_bf[:], start=True, stop=True)
                nc.vector.tensor_tensor(out=dst, in0=otmp[base:base + HD, :],
                                        in1=rbp[base:base + HD, :],
                                        op=ALU.mult)

            def emit_fw(sc, ets):
                """Fused FFN2 + W_o for out chunks ets at query cols sc.

                W2 is host-scaled by WS (bf16, exact) and W_o is fp8 scaled
                by WS, so both accumulate WS*value into the same PSUM bank;
                the final copy multiplies by 1/WS.
                """
                ob = obp.tile([P, 2, 512], BF16, tag="ob")
                for k, et in enumerate(ets):
                    po = fwps.tile([P, 512], F32, tag="fw")
                    for c in range(DC):
                        nc.tensor.matmul(po[:], lhsT=w2_sb[:, et, c, :],
                                         rhs=hid[:, c, ds(512 * sc, 512)],
                                         start=(c == 0), stop=False)
                    nc.tensor.matmul(po[:], lhsT=wo_sb[:, et],
                                     rhs=ot[:, sc],
                                     start=False, stop=True, perf_mode=DR)
                    nc.vector.tensor_scalar_mul(ob[:, k, :], po[:],
                                                 1.0 / WS)
                nc.sync.dma_start(
                    out_r[:, ds(ets[0], 2), ds(512 * sc, 512)], ob[:])

            # all of FFN1 first: keeps every Gelu ACTIVATE ahead (in scheduler
            # priority) of every attention Exp, so the ACT table swaps once.

            # attention; FW(sc-1) interleaved into sc's head loop
            for sc in range(4):
                for h in range(NH):
                    emit_head(sc, h)
                    if sc > 0:
                        emit_fw(sc - 1, [2 * h, 2 * h + 1])
            for j in range(4):
                emit_fw(3, [2 * j, 2 * j + 1])


def build(gelu_func=None):
    if gelu_func is None:
        gelu_func = AF.Gelu
    nc = bacc.Bacc("TRN2", target_bir_lowering=False, debug=False,
                   enable_asserts=True, num_devices=NCORES)
    aps = {}

    def din(name, shape, dtype):
        aps[name] = nc.dram_tensor(name, list(shape), dtype,
                                   kind="ExternalInput").ap()

    din("x", (S, D), BF16)
    din("wqkva", (P, 4, 2, 512), F8)
    din("wqkvb", (P, 4, 2, 256), F8)
    din("bqkv", (P, 3 * DSH), F32)
    din("wo", (P, DC, 2, P), F8)
    din("w1", (P, DC, DC, P), BF16)
    din("b1p", (P, DC), F32)
    din("w2", (P, DC, DC, P), BF16)
    din("cosr", (P, NT, RH), BF16)
    din("sinr", (P, NT, RH), BF16)
    din("maskd", (P, P), F8)
    aps["outp"] = nc.dram_tensor("outp", [D, S], BF16,
                                 kind="ExternalOutput").ap()

    with tile.TileContext(nc) as tc:
        _body(tc, aps, gelu_func)
    nc.compile()
    return nc


def make_in_maps(inputs):
    x = np.asarray(inputs["x"], np.float32)
    Wqkv = np.asarray(inputs["W_qkv"], np.float32)
    b_qkv = np.asarray(inputs["b_qkv"], np.float32)
    Wo = np.asarray(inputs["W_o"], np.float32)
    ln1w = np.asarray(inputs["ln1_w"], np.float32)
    ln1b = np.asarray(inputs["ln1_b"], np.float32)
    ln2w = np.asarray(inputs["ln2_w"], np.float32)
    ln2b = np.asarray(inputs["ln2_b"], np.float32)
    W1 = np.asarray(inputs["W1"], np.float32)
    b1 = np.asarray(inputs["b1"], np.float32)
    W2 = np.asarray(inputs["W2"], np.float32)
    freqs = np.asarray(inputs["freqs_cis"], np.float32)

    cos = freqs[0, 0, :, :, 0]
    sin = freqs[0, 0, :, :, 1]
    cosr = np.ascontiguousarray(
        cos.reshape(NT, P, RH).transpose(1, 0, 2)).astype(bf16)
    sinr = np.ascontiguousarray(
        sin.reshape(NT, P, RH).transpose(1, 0, 2)).astype(bf16)
    kq = np.arange(P)
    maskd = (kq[:, None] <= kq[None, :]).astype(f8e4)

    in_maps = []
    for core in range(NCORES):
        b = core // 4
        r = core % 4
        sl = slice(256 * r, 256 * r + 256)
        Ws = np.concatenate([Wqkv[0:D][sl], Wqkv[D:2 * D][sl],
                             Wqkv[2 * D:3 * D][sl]], 0)          # [768, 1024]
        bq = np.concatenate([b_qkv[0:D][sl], b_qkv[D:2 * D][sl],
                             b_qkv[2 * D:3 * D][sl]], 0)
        Wsp = Ws * ln1w[None, :]
        bqp = (bq + Ws @ ln1b).astype(np.float32)
        # fp8 DoubleRow layout [p, kp, j, out]: contraction d = (2kp+j)*128+p
        wq8 = (Wsp.T * 64.0).reshape(4, 2, P, 3 * DSH).transpose(2, 0, 1, 3)
        wqkva_l = np.ascontiguousarray(wq8[:, :, :, 0:512]).astype(f8e4)
        wqkvb_l = np.ascontiguousarray(wq8[:, :, :, 512:768]).astype(f8e4)
        bqkv_l = np.ascontiguousarray(
            np.broadcast_to(bqp[None, :], (P, 3 * DSH))).astype(np.float32)
        Wos = Wo[:, sl]                                           # [1024, 256]
        wo_l = np.ascontiguousarray(
            (Wos.T * 64.0).reshape(2, P, DC, P)
            .transpose(1, 2, 0, 3)).astype(f8e4)
        W1s = W1[FSH * r: FSH * (r + 1)]                          # [1024, 1024]
        W1p = W1s * ln2w[None, :]
        b1p = (b1[FSH * r: FSH * (r + 1)] + W1s @ ln2b).astype(np.float32)
        w1_l = np.ascontiguousarray(
            W1p.reshape(DC, P, DC, P).transpose(3, 0, 2, 1)).astype(bf16)
        b1_l = np.ascontiguousarray(b1p.reshape(DC, P).T).astype(np.float32)
        W2s = W2[:, FSH * r: FSH * (r + 1)] * 64.0                # [1024, 1024]
        w2_l = np.ascontiguousarray(
            W2s.reshape(DC, P, DC, P).transpose(3, 0, 2, 1)).astype(bf16)
        in_maps.append(dict(
            x=np.ascontiguousarray(x[b]).astype(bf16),
            wqkva=wqkva_l, wqkvb=wqkvb_l,
            bqkv=bqkv_l, wo=wo_l,
            w1=w1_l, b1p=b1_l, w2=w2_l, cosr=cosr, sinr=sinr, maskd=maskd))
    return in_maps


def gather(inputs, results):
    x = np.asarray(inputs["x"], np.float32)
    bias = (np.asarray(inputs["b_o"], np.float32)
            + np.asarray(inputs["b2"], np.float32))
    outs = [np.asarray(res["outp"], np.float32) for res in results]
    out = np.empty((B, S, D), np.float32)
    for b in range(B):
        acc = outs[4 * b] + outs[4 * b + 1] + outs[4 * b + 2] + outs[4 * b + 3]
        out[b] = x[b] + acc.T + bias[None, :]
    return out


_CACHE = {}


def kernel(**inputs):
    if "nc" not in _CACHE:
        _CACHE["nc"] = build()
    nc = _CACHE["nc"]
    in_maps = make_in_maps(inputs)
    res = bass_utils.run_bass_kernel_spmd(nc, in_maps,
                                          core_ids=list(range(NCORES)))
    return gather(inputs, res.results)


# revision 40
# speedup vs baseline: 1.0662x; 1.0259x over previous
"""Trainium2 Bass kernel for a GPT-J-style (parallel-residual) decoder layer.

Problem: B=2, S=2048, D=1024, H=16 heads x 64, rotary_dim=16, FF=4096, causal.

Sharding (8 NeuronCores): data-parallel over batch (2) x tensor-parallel over
heads/FFN (4).  Core c handles batch c//4 and TP rank r=c%4: heads 4r..4r+3
(256 of the 1024 attention dims), FFN rows 1024r..1024r+1024.
LayerNorm affine params are folded into the weights on the host, so the device
computes a single normalized activation xhat shared by attention and FFN.
Each core returns partial^T = (attn_partial + ffn_partial)^T in [D, S] bf16;
the host sums the 4 TP partials per batch and adds x + b_o + b2.

Schedule (v1): fully pipelined.
  P1: per 4-tile group: DMA x -> LN stats -> batched sqrt -> xhat ->
      PE-transpose (identity matmul) -> QKV matmuls; rotary + PE-transpose of
      q,k to e-major per half.  No DRAM staging anywhere.
  P2/P3: attention is query-chunk-outer (sc = 512 cols); FFN1+GELU emitted
      after attention sc0 (single ACT-table swap Exp->Gelu->Exp);
      FFN2 and W_o accumulate into the SAME PSUM bank per (et, sc) right
      after each sc's heads finish, interleaved into the next sc's head loop;
      outputs stream to DRAM per 2-et chunk in bf16.
"""

import numpy as np
import ml_dtypes

import concourse.bass as bass
import concourse.mybir as mybir
import concourse.tile as tile
import concourse.bass_utils as bass_utils
from concourse import bacc
from concourse import masks
from concourse.bass import ds, ts

B, S, D = 2, 2048, 1024
H, HD = 16, 64
ROT, RH = 16, 8
FF = 4096
EPS = 1e-5
P = 128
NT = S // P            # 16 sequence tiles
DC = D // P            # 8 model-dim chunks
NH = 4                 # heads per core
DSH = NH * HD          # 256 attention dims per core
FSH = FF // 4          # 1024 FFN rows per core
NCORES = 8

F32 = mybir.dt.float32
BF16 = mybir.dt.bfloat16
F8 = mybir.dt.float8e4
DR = mybir.MatmulPerfMode.DoubleRow
AF = mybir.ActivationFunctionType
ALU = mybir.AluOpType
bf16 = ml_dtypes.bfloat16
f8e4 = ml_dtypes.float8_e4m3fn
WS = 64.0   # fp8 weight scale (power of 2; psum carries WS*value)


def _body(tc, aps, gelu_func):
    nc = tc.nc
    x_d = aps["x"].rearrange("(t p) d -> p t d", p=P)        # [128, 16, 1024]
    bqkv_d = aps["bqkv"]
    wo_d = aps["wo"]
    w1_d = aps["w1"]
    b1_d = aps["b1p"]
    w2_d = aps["w2"]
    cos_d = aps["cosr"]
    sin_d = aps["sinr"]
    mask_d = aps["maskd"]
    out_r = aps["outp"].rearrange("(c p) s -> p c s", p=P)   # [128, 8, 2048]

    with (
        tc.tile_pool(name="const", bufs=1) as const,
        tc.tile_pool(name="big", bufs=1) as big,
    ):
        # ---- persistent SBUF: weights + activations ----
        # x tiles stream first (alternating the two hwdge queues); weight
        # loads are emitted after so they don't delay the LN pipeline.
        # w1 is deferred until the FFN1 phase.
        xall = const.tile([P, NT, D], BF16)                  # 32KB/part
        wqkva_sb = const.tile([P, 4, 2, 512], F8)
        wqkvb_sb = const.tile([P, 4, 2, 256], F8)
        bqkv_sb = const.tile([P, 3 * DSH], F32)
        wo_sb = const.tile([P, DC, 2, P], F8)
        b1_sb = const.tile([P, DC], F32)
        cos_sb = const.tile([P, NT, RH], BF16)
        sin_sb = const.tile([P, NT, RH], BF16)
        mask_sb = const.tile([P, P], F8)
        w1_sb = const.tile([P, DC, DC, P], BF16)             # 16KB/part

        for t in range(NT):
            eng = nc.sync if t % 2 == 0 else nc.scalar
            if t == 0:
                # halves: LN stats for tile 0 start after the first 512
                nc.sync.dma_start(xall[:, 0, 0:512], x_d[:, 0, 0:512])
                nc.sync.dma_start(xall[:, 0, 512:D], x_d[:, 0, 512:D])
            else:
                eng.dma_start(xall[:, t, :], x_d[:, t, :])
            # weave the QKV-critical weights between the first x tiles on
            # the scalar hwdge queue so QKV/bias never wait on them
            if t == 1:
                nc.scalar.dma_start(wqkva_sb[:], aps["wqkva"])
            elif t == 3:
                nc.scalar.dma_start(wqkvb_sb[:], aps["wqkvb"])
            elif t == 5:
                nc.scalar.dma_start(bqkv_sb[:], bqkv_d)

        def load_weights():
            # weights not needed until FFN1 / rotary / stage C
            nc.scalar.dma_start(b1_sb[:], b1_d)
            for ft in range(DC):
                nc.scalar.dma_start(w1_sb[:, ft], w1_d[:, ft])
            nc.scalar.dma_start(wo_sb[:], wo_d)
            nc.scalar.dma_start(cos_sb[:], cos_d)
            nc.scalar.dma_start(sin_sb[:], sin_d)
            nc.scalar.dma_start(mask_sb[:], mask_d)
        eps_sb = const.tile([P, 1], F32)
        nc.vector.memset(eps_sb[:], EPS)
        warm = const.tile([P, 1], F32)
        nc.scalar.activation(warm[:], eps_sb[:], AF.Sqrt)   # preload Sqrt table
        ones_sb = const.tile([1, HD], BF16)
        nc.vector.memset(ones_sb[:], 1.0)
        ident = const.tile([P, P], BF16)
        masks.make_identity(nc, ident[:])
        mvall = const.tile([P, NT, 2], F32)                  # LN mean/var

        xhatT = big.tile([P, DC, S], BF16)          # xhat dim-major [d, s]
        xhatT8 = big.tile([P, 4, NT, 2, P], F8)     # fp8, kp-pair packed
        vp = big.tile([P, NT // 2, NH, 2, HD + 16], F8)  # v pair-packed+ones
        # inner width 80 = 16B-aligned even stride (dual-fp8 LDW restriction)
        qe = big.tile([P, 2, S], BF16)              # q e-major
        ke = big.tile([P, 2, S], BF16)              # k e-major
        ot = big.tile([P, 4, 2, 512], F8)           # attn out, sc-major
        hid = big.tile([P, DC, S], BF16)            # ffn hidden, f-major

        nc.vector.memset(vp[:, :, :, :, HD:HD + 1], 1.0)

        # ================= P1: LN + transpose + QKV + rotary =================
        with (
            tc.tile_pool(name="qkp", bufs=1) as qkp,
            tc.tile_pool(name="statp", bufs=8) as statp,
            tc.tile_pool(name="xhp", bufs=3) as xhp,
            tc.tile_pool(name="rotp", bufs=3) as rotp,
            tc.tile_pool(name="tpps", bufs=2, space="PSUM") as tpps,
            tc.tile_pool(name="qtps", bufs=1, space="PSUM") as qtps,
            tc.tile_pool(name="qaps", bufs=2, space="PSUM") as qaps,
            tc.tile_pool(name="qbps", bufs=1, space="PSUM") as qbps,
            tc.tile_pool(name="ff1ps", bufs=2, space="PSUM") as ff1ps,
        ):
            qk = qkp.tile([P, NT, 2 * DSH], BF16)   # q,k token-major (scoped)

            def emit_ffn1(sc):
                for ft in range(DC):
                    ps = ff1ps.tile([P, 512], F32, tag="f1")
                    for c in range(DC):
                        nc.tensor.matmul(ps[:], lhsT=w1_sb[:, ft, c, :],
                                         rhs=xhatT[:, c, ds(512 * sc, 512)],
                                         start=(c == 0), stop=(c == DC - 1))
                    nc.scalar.activation(hid[:, ft, ds(512 * sc, 512)],
                                         ps[:], gelu_func,
                                         bias=b1_sb[:, ft:ft + 1])

            def do_rotary(half):
                cosb = cos_sb[:, ds(8 * half, 8), :].unsqueeze(2) \
                    .to_broadcast([P, 8, NH, RH])
                sinb = sin_sb[:, ds(8 * half, 8), :].unsqueeze(2) \
                    .to_broadcast([P, 8, NH, RH])
                for part in range(2):   # 0: q, 1: k
                    sl = qk[:, ds(8 * half, 8), ds(DSH * part, DSH)].rearrange(
                        "p t (h e) -> p t h e", h=NH)
                    x1 = sl[:, :, :, 0:RH]
                    x2 = sl[:, :, :, RH:ROT]
                    t1 = rotp.tile([P, 8, NH, RH], BF16, tag="rt")
                    t2 = rotp.tile([P, 8, NH, RH], BF16, tag="rt")
                    t3 = rotp.tile([P, 8, NH, RH], BF16, tag="rt")
                    nc.vector.tensor_tensor(out=t1[:], in0=x1, in1=cosb,
                                            op=ALU.mult)
                    nc.vector.tensor_tensor(out=t2[:], in0=x2, in1=sinb,
                                            op=ALU.mult)
                    nc.vector.tensor_tensor(out=t1[:], in0=t1[:], in1=t2[:],
                                            op=ALU.subtract)
                    nc.vector.tensor_tensor(out=t2[:], in0=x1, in1=sinb,
                                            op=ALU.mult)
                    nc.vector.tensor_tensor(out=t3[:], in0=x2, in1=cosb,
                                            op=ALU.mult)
                    nc.vector.tensor_tensor(out=t2[:], in0=t2[:], in1=t3[:],
                                            op=ALU.add)
                    nc.vector.tensor_copy(out=x1, in_=t1[:])
                    nc.vector.tensor_copy(out=x2, in_=t2[:])

            def qk_transpose(t):
                # q,k of tile t -> e-major qe/ke columns [t*128, t*128+128)
                qt = qtps.tile([P, 4, P], BF16, tag="qt")
                for j in range(4):
                    nc.tensor.transpose(qt[:, j, :], qk[:, t, ds(P * j, P)],
                                        ident[:])
                nc.scalar.activation(qe[:, :, ts(t, P)], qt[:, 0:2, :],
                                     AF.Copy)
                nc.scalar.activation(ke[:, :, ts(t, P)], qt[:, 2:4, :],
                                     AF.Copy)

            for g in range(4):
                if g == 0:
                    # group 0 pipelines per-tile so tile 0's chain has the
                    # lowest scheduler priority (earliest execution)
                    rstds = []
                    def stats_tile(t):
                        st = statp.tile([P, 2, 6], F32, tag="st")
                        xr = xall[:, t, :].rearrange("p (a f) -> p a f",
                                                     f=512)
                        for sg in range(2):
                            nc.vector.bn_stats(st[:, sg, :], xr[:, sg, :])
                        nc.vector.bn_aggr(mvall[:, t, :], st[:])
                        sd = statp.tile([P, 1], F32, tag=f"s{t}")
                        nc.scalar.activation(sd[:], mvall[:, ds(t, 1), 1],
                                             AF.Sqrt, bias=eps_sb[:])
                        rs = statp.tile([P, 1], F32, tag=f"r{t}")
                        nc.vector.reciprocal(rs[:], sd[:])
                        rstds.append(rs)
                    rstd_of = lambda j: rstds[j][:, 0:1]
                else:
                    for j in range(4):
                        t = 4 * g + j
                        st = statp.tile([P, 2, 6], F32, tag="st")
                        xr = xall[:, t, :].rearrange("p (a f) -> p a f",
                                                     f=512)
                        for sg in range(2):
                            nc.vector.bn_stats(st[:, sg, :], xr[:, sg, :])
                        nc.vector.bn_aggr(mvall[:, t, :], st[:])
                    stdg = statp.tile([P, 4], F32, tag="sd", name=f"sd{g}")
                    nc.scalar.activation(stdg[:], mvall[:, ds(4 * g, 4), 1],
                                         AF.Sqrt, bias=eps_sb[:])
                    rstdg = statp.tile([P, 4], F32, tag="rs", name=f"rs{g}")
                    nc.vector.reciprocal(rstdg[:], stdg[:])
                    rstd_of = lambda j, r=rstdg: r[:, ds(j, 1)]
                for j in range(4):
                    t = 4 * g + j
                    if g == 0:
                        stats_tile(t)
                        if j == 3:
                            load_weights()
                    xh = xhp.tile([P, D], BF16, tag="xh")
                    nc.vector.tensor_scalar(out=xh[:], in0=xall[:, t, :],
                                            scalar1=mvall[:, t, 0:1],
                                            scalar2=rstd_of(j),
                                            op0=ALU.subtract, op1=ALU.mult)
                    tp = tpps.tile([P, DC, P], BF16, tag="tp")
                    for c in range(DC):
                        nc.tensor.transpose(tp[:, c, :], xh[:, ds(P * c, P)],
                                            ident[:])
                    nc.scalar.activation(
                        xhatT8[:, :, t, :, :],
                        tp[:].rearrange("p (a b) c -> p a b c", b=2),
                        AF.Copy)
                    nc.vector.tensor_copy(out=xhatT[:, :, ts(t, P)], in_=tp[:])
                    # QKV for tile t (fp8 DoubleRow: K=256 per pass)
                    psa = qaps.tile([P, 512], F32, tag="qa")
                    psb = qbps.tile([P, 256], F32, tag="qb")
                    for kp in range(4):
                        l = xhatT8[:, kp, t, :, :]
                        nc.tensor.matmul(psa[:], lhsT=l,
                                         rhs=wqkva_sb[:, kp],
                                         start=(kp == 0), stop=(kp == 3),
                                         perf_mode=DR)
                        nc.tensor.matmul(psb[:], lhsT=l,
                                         rhs=wqkvb_sb[:, kp],
                                         start=(kp == 0), stop=(kp == 3),
                                         perf_mode=DR)
                    nc.vector.scalar_tensor_tensor(
                        out=qk[:, t, :], in0=psa[:], scalar=1.0 / WS,
                        in1=bqkv_sb[:, 0:512], op0=ALU.mult, op1=ALU.add)
                    nc.vector.scalar_tensor_tensor(
                        out=vp[:, t // 2, :, t % 2, 0:HD],
                        in0=psb[:].rearrange("p (h e) -> p h e", h=NH),
                        scalar=1.0 / WS,
                        in1=bqkv_sb[:, 512:768].rearrange(
                            "p (h e) -> p h e", h=NH),
                        op0=ALU.mult, op1=ALU.add)
                if g % 2 == 1:
                    half = g // 2
                    do_rotary(half)
                    for t in range(8 * half, 8 * half + 8):
                        qk_transpose(t)
                emit_ffn1(g)

        # ============ P2/P3: attention (sc-outer) + FFN1 + fused FFN2/WO ====
        with (
            tc.tile_pool(name="w2p", bufs=1) as w2p,
            tc.tile_pool(name="ptp", bufs=4) as ptp,
            tc.tile_pool(name="sump", bufs=2) as sump,
            tc.tile_pool(name="obp", bufs=2) as obp,
            tc.tile_pool(name="scps", bufs=3, space="PSUM") as scps,
            tc.tile_pool(name="ovps", bufs=2, space="PSUM") as ovps,
            tc.tile_pool(name="fwps", bufs=3, space="PSUM") as fwps,
        ):
            w2_sb = w2p.tile([P, DC, DC, P], BF16)           # 16KB/part
            for et in range(DC):
                nc.sync.dma_start(w2_sb[:, et], w2_d[:, et])

            def emit_head(sc, h):
                """Attention for query cols [512sc, 512sc+512), head h.

                Key tiles are processed in pairs (i0, i1 = 2k, 2k+1): exp
                probabilities go to the two fp8 planes of pt2 and one
                DoubleRow matmul contracts both tiles (256 keys) at once.
                """
                base = HD * (h % 2)
                cix = h // 2
                ov = ovps.tile([HD + 1, 512], F32, tag="ov",
                               name=f"ov_{sc}_{h}")
                klast = 2 * sc + 1
                for k in range(klast + 1):
                    i0, i1 = 2 * k, 2 * k + 1
                    lo0 = max(512 * sc, P * i0)
                    lo1 = max(512 * sc, P * i1)
                    wid0 = 512 * (sc + 1) - lo0      # 512 or 256
                    wid1 = 512 * (sc + 1) - lo1
                    o0 = lo0 - 512 * sc              # ov-relative offset
                    rel1 = lo1 - lo0                 # plane-1 start in pt2
                    pt2 = ptp.tile([P, 2, wid0], F8, tag=f"pt{wid0}")
                    for j, (i, lo, wid, rel) in enumerate(
                            ((i0, lo0, wid0, 0), (i1, lo1, wid1, rel1))):
                        ps = scps.tile([P, 512], F32, tag="sc")
                        nc.tensor.matmul(
                            ps[:, :wid],
                            lhsT=ke[base:base + HD, cix, ts(i, P)],
                            rhs=qe[base:base + HD, cix, ds(lo, wid)],
                            start=True, stop=True)
                        nc.scalar.activation(pt2[:, j, ds(rel, wid)],
                                             ps[:, :wid], AF.Exp, scale=0.125)
                        if lo == P * i:   # diagonal block: causal mask
                            nc.vector.tensor_tensor(
                                out=pt2[:, j, ds(rel, P)],
                                in0=pt2[:, j, ds(rel, P)],
                                in1=mask_sb[:], op=ALU.mult)
                    if rel1 > 0:   # plane 1 has no keys for cols [lo0, lo1)
                        nc.vector.memset(pt2[:, 1, ds(0, rel1)], 0.0)
                    nc.tensor.matmul(
                        ov[:, ds(o0, wid0)],
                        lhsT=vp[:, k, h, :, 0:HD + 1],
                        rhs=pt2[:],
                        start=(k == 0), stop=(k == klast),
                        perf_mode=DR)
                # renormalize: ot = ov[:HD] * (1/rowsum) broadcast via PE
                dst = ot[base:base + HD, sc, cix, :]
                otmp = sump.tile([P, 512], BF16, tag="ot", name=f"ot_{sc}_{h}")
                nc.vector.tensor_copy(out=otmp[base:base + HD, :],
                                      in_=ov[0:HD, :])
                sume = sump.tile([1, 512], F32, tag="se", name=f"se_{sc}_{h}")
                nc.vector.tensor_copy(out=sume[:], in_=ov[HD:HD + 1, :])
                rinv = sump.tile([1, 512], F32, tag="ri", name=f"ri_{sc}_{h}")
                nc.vector.reciprocal_approx_fast(out=rinv[:], in_=sume[:])
                rinv_bf = sump.tile([1, 512], BF16, tag="rb",
                                    name=f"rb_{sc}_{h}")
                nc.vector.tensor_copy(out=rinv_bf[:], in_=rinv[:])
                rbp = ovps.tile([P, 512], F32, tag="ov", name=f"rp_{sc}_{h}")
                nc.tensor.matmul(rbp[base:base + HD, :], lhsT=ones_sb[:],
                                 rhs=rinv_bf[:], start=True, stop=True)
                nc.vector.tensor_tensor(out=dst, in0=otmp[base:base + HD, :],
                                        in1=rbp[base:base + HD, :],
                                        op=ALU.mult)

            def emit_fw(sc, ets):
                """Fused FFN2 + W_o for out chunks ets at query cols sc.

                W2 is host-scaled by WS (bf16, exact) and W_o is fp8 scaled
                by WS, so both accumulate WS*value into the same PSUM bank;
                the final copy multiplies by 1/WS.
                """
                ob = obp.tile([P, len(ets), 512], BF16,
                              tag=f"ob{len(ets)}")
                for k, et in enumerate(ets):
                    po = fwps.tile([P, 512], F32, tag="fw")
                    for c in range(DC):
                        nc.tensor.matmul(po[:], lhsT=w2_sb[:, et, c, :],
                                         rhs=hid[:, c, ds(512 * sc, 512)],
                                         start=(c == 0), stop=False)
                    nc.tensor.matmul(po[:], lhsT=wo_sb[:, et],
                                     rhs=ot[:, sc],
                                     start=False, stop=True, perf_mode=DR)
                    nc.vector.tensor_scalar_mul(ob[:, k, :], po[:],
                                                 1.0 / WS)
                nc.sync.dma_start(
                    out_r[:, ds(ets[0], len(ets)), ds(512 * sc, 512)], ob[:])

            # all of FFN1 first: keeps every Gelu ACTIVATE ahead (in scheduler
            # priority) of every attention Exp, so the ACT table swaps once.

            # attention; FW(sc-1) interleaved into sc's head loop
            for sc in range(4):
                for h in range(NH):
                    emit_head(sc, h)
                    if sc > 0:
                        emit_fw(sc - 1, [2 * h, 2 * h + 1])
            for j in range(3):
                emit_fw(3, [2 * j, 2 * j + 1])
            emit_fw(3, [6])
            emit_fw(3, [7])


def build(gelu_func=None):
    if gelu_func is None:
        gelu_func = AF.Gelu
    nc = bacc.Bacc("TRN2", target_bir_lowering=False, debug=False,
                   enable_asserts=True, num_devices=NCORES)
    aps = {}

    def din(name, shape, dtype):
        aps[name] = nc.dram_tensor(name, list(shape), dtype,
                                   kind="ExternalInput").ap()

    din("x", (S, D), BF16)
    din("wqkva", (P, 4, 2, 512), F8)
    din("wqkvb", (P, 4, 2, 256), F8)
    din("bqkv", (P, 3 * DSH), F32)
    din("wo", (P, DC, 2, P), F8)
    din("w1", (P, DC, DC, P), BF16)
    din("b1p", (P, DC), F32)
    din("w2", (P, DC, DC, P), BF16)
    din("cosr", (P, NT, RH), BF16)
    din("sinr", (P, NT, RH), BF16)
    din("maskd", (P, P), F8)
    aps["outp"] = nc.dram_tensor("outp", [D, S], BF16,
                                 kind="ExternalOutput").ap()

    with tile.TileContext(nc) as tc:
        _body(tc, aps, gelu_func)
    nc.compile()
    return nc


def make_in_maps(inputs):
    x = np.asarray(inputs["x"], np.float32)
    Wqkv = np.asarray(inputs["W_qkv"], np.float32)
    b_qkv = np.asarray(inputs["b_qkv"], np.float32)
    Wo = np.asarray(inputs["W_o"], np.float32)
    ln1w = np.asarray(inputs["ln1_w"], np.float32)
    ln1b = np.asarray(inputs["ln1_b"], np.float32)
    ln2w = np.asarray(inputs["ln2_w"], np.float32)
    ln2b = np.asarray(inputs["ln2_b"], np.float32)
    W1 = np.asarray(inputs["W1"], np.float32)
    b1 = np.asarray(inputs["b1"], np.float32)
    W2 = np.asarray(inputs["W2"], np.float32)
    freqs = np.asarray(inputs["freqs_cis"], np.float32)

    cos = freqs[0, 0, :, :, 0]
    sin = freqs[0, 0, :, :, 1]
    cosr = np.ascontiguousarray(
        cos.reshape(NT, P, RH).transpose(1, 0, 2)).astype(bf16)
    sinr = np.ascontiguousarray(
        sin.reshape(NT, P, RH).transpose(1, 0, 2)).astype(bf16)
    kq = np.arange(P)
    maskd = (kq[:, None] <= kq[None, :]).astype(f8e4)

    in_maps = []
    for core in range(NCORES):
        b = core // 4
        r = core % 4
        sl = slice(256 * r, 256 * r + 256)
        Ws = np.concatenate([Wqkv[0:D][sl], Wqkv[D:2 * D][sl],
                             Wqkv[2 * D:3 * D][sl]], 0)          # [768, 1024]
        bq = np.concatenate([b_qkv[0:D][sl], b_qkv[D:2 * D][sl],
                             b_qkv[2 * D:3 * D][sl]], 0)
        Wsp = Ws * ln1w[None, :]
        bqp = (bq + Ws @ ln1b).astype(np.float32)
        # fp8 DoubleRow layout [p, kp, j, out]: contraction d = (2kp+j)*128+p
        wq8 = (Wsp.T * 64.0).reshape(4, 2, P, 3 * DSH).transpose(2, 0, 1, 3)
        wqkva_l = np.ascontiguousarray(wq8[:, :, :, 0:512]).astype(f8e4)
        wqkvb_l = np.ascontiguousarray(wq8[:, :, :, 512:768]).astype(f8e4)
        bqkv_l = np.ascontiguousarray(
            np.broadcast_to(bqp[None, :], (P, 3 * DSH))).astype(np.float32)
        Wos = Wo[:, sl]                                           # [1024, 256]
        wo_l = np.ascontiguousarray(
            (Wos.T * 64.0).reshape(2, P, DC, P)
            .transpose(1, 2, 0, 3)).astype(f8e4)
        W1s = W1[FSH * r: FSH * (r + 1)]                          # [1024, 1024]
        W1p = W1s * ln2w[None, :]
        b1p = (b1[FSH * r: FSH * (r + 1)] + W1s @ ln2b).astype(np.float32)
        w1_l = np.ascontiguousarray(
            W1p.reshape(DC, P, DC, P).transpose(3, 0, 2, 1)).astype(bf16)
        b1_l = np.ascontiguousarray(b1p.reshape(DC, P).T).astype(np.float32)
        W2s = W2[:, FSH * r: FSH * (r + 1)] * 64.0                # [1024, 1024]
        w2_l = np.ascontiguousarray(
            W2s.reshape(DC, P, DC, P).transpose(3, 0, 2, 1)).astype(bf16)
        in_maps.append(dict(
            x=np.ascontiguousarray(x[b]).astype(bf16),
            wqkva=wqkva_l, wqkvb=wqkvb_l,
            bqkv=bqkv_l, wo=wo_l,
            w1=w1_l, b1p=b1_l, w2=w2_l, cosr=cosr, sinr=sinr, maskd=maskd))
    return in_maps


def gather(inputs, results):
    x = np.asarray(inputs["x"], np.float32)
    bias = (np.asarray(inputs["b_o"], np.float32)
            + np.asarray(inputs["b2"], np.float32))
    outs = [np.asarray(res["outp"], np.float32) for res in results]
    out = np.empty((B, S, D), np.float32)
    for b in range(B):
        acc = outs[4 * b] + outs[4 * b + 1] + outs[4 * b + 2] + outs[4 * b + 3]
        out[b] = x[b] + acc.T + bias[None, :]
    return out


_CACHE = {}


def kernel(**inputs):
    if "nc" not in _CACHE:
        _CACHE["nc"] = build()
    nc = _CACHE["nc"]
    in_maps = make_in_maps(inputs)
    res = bass_utils.run_bass_kernel_spmd(nc, in_maps,
                                          core_ids=list(range(NCORES)))
    return gather(inputs, res.results)
